# revision 12
# baseline (speedup 1.0000x reference)
"""Causal self-attention (B=2, T=2048, C=1024, 16 heads) on 8 TRN2 NeuronCores.

Sharding: core = b*4 + hg  (b in {0,1} data-parallel over batch,
hg in {0..3} tensor-parallel over head groups of 4 heads).
Each core computes QKV projection for its 4 heads, causal attention, and a
partial output projection (its 256 rows of w_proj); the host sums the 4
partials per batch element and adds b_proj (the tensor-parallel all-reduce).

Device kernel design (per core):
- x / weights arrive as bf16 (halves HBM traffic; QKV+proj matmuls run
  bf16 at 1 cycle/row, PSUM accumulates fp32).
- q,k produced transposed (channels x T); k packed per head-pair on
  partition halves (h_even on 0-63, h_odd on 64-127).
- Scores S^T computed per HEAD-PAIR with row-tiled K=64 matmuls:
  tile_position (0,0)/(64,0) auto-derived from the operands' base
  partition; the two matmuls run CONCURRENTLY in the PE array (measured
  110ns/MM vs 202ns/MM for serialized K=128) - this replaces the old
  zero-pad-K-to-128 scheme at ~2x the S throughput.
- s tile [128, 1024] holds both heads (h0 cols 0-511, h1 512-1023), so
  exp on ScalarE batches both heads in one activation for off-diagonal
  blocks. Causal masking via restricted column ranges + triangular-mask
  multiplies on diagonal blocks.
- v natural (T x ch) with a ones-column FIRST per head so a single AV
  matmul also accumulates the softmax denominator at PSUM partition 0
  (lhsT = [1 | 63 pad | v], M=128; channels land at PSUM partition 64 -
  wide PSUM APs must be 64-aligned).
- Softmax normalization: denominator at PSUM partition 0, DVE reciprocal
  straight from PSUM, GpSimd partition_broadcast, one DVE multiply
  writing bf16 attn (the proj lhsT).
- Output projection epilogue: PSUM->SBUF copies split ScalarE/DVE, y
  written bf16, one DMA per 128-row tile. b_proj added on the host.
- Inputs land in wide multi-dim DMAs spread over FOUR rings (SP +
  Activation HWDGE, Vector + GpSimd SWDGE); consts/tri land first.
- ~10 warmup matmuls on the first-landed xt columns keep the PE busy
  during the input-DMA window so the HAM clock gate reaches K=8/8
  before the real work starts (cold PE runs at 1.2 instead of 2.4 GHz).
"""
import numpy as np
from contextlib import ExitStack

import ml_dtypes

import concourse.bass as bass
import concourse.tile as tile
from concourse import bacc, mybir
from concourse.bass_utils import run_bass_kernel_spmd

F32 = mybir.dt.float32
F32R = mybir.dt.float32r
BF16 = mybir.dt.bfloat16
AF = mybir.ActivationFunctionType

B, T, C = 2, 2048, 1024
N_HEAD, HEAD_DIM = 16, 64
N_CORES = 8
H_LOC = 4          # heads per core
CQK = 512          # local q+k channels (4 heads * 64 * 2)
CV = 256           # local v channels
KT = 8             # contraction tiles over C (1024/128)
NTQ = 4            # T blocks of 512 (queries)
NT16 = 16          # T blocks of 128
SCALE = 1.0 / 8.0  # 1/sqrt(HEAD_DIM)

# consts layout (one packed [128, 262] f32 tensor)
CO_BQK = 0     # [128, 4]   qkv bias, one col per co block
CO_KM = 4      # [128, 2]   (unused)
CO_BV = 6      # [128, 256] v bias broadcast
NCONST = 262

_cached_nc = None


def _build():
    nc = bacc.Bacc("TRN2", target_bir_lowering=False, debug=False,
                   enable_asserts=False, num_devices=N_CORES)
    # All inputs arrive pre-relayouted host-side to match their SBUF tile
    # layout exactly, so every load is a plain 2D contiguous DMA.
    xt = nc.dram_tensor("xt", [128, KT * T], BF16, kind="ExternalInput").ap()
    wqk = nc.dram_tensor("wqk", [128, KT * CQK], BF16, kind="ExternalInput").ap()
    wv = nc.dram_tensor("wv", [128, KT * CV], BF16, kind="ExternalInput").ap()
    wp = nc.dram_tensor("wp", [128, 2 * C], BF16, kind="ExternalInput").ap()
    consts = nc.dram_tensor("consts", [128, NCONST], F32, kind="ExternalInput").ap()
    trib = nc.dram_tensor("trib", [128, 128], BF16, kind="ExternalInput").ap()
    y = nc.dram_tensor("y", [T, C], BF16, kind="ExternalOutput").ap()

    with tile.TileContext(nc) as tc, ExitStack() as ctx:
        big = ctx.enter_context(tc.tile_pool(name="big", bufs=1))
        work = ctx.enter_context(tc.tile_pool(name="work", bufs=2))
        psum = ctx.enter_context(tc.tile_pool(name="psum", bufs=1, space="PSUM"))

        # ---- persistent SBUF tensors ----
        xt_sb = big.tile([128, KT * T], BF16, tag="xt")        # 32KB/p
        wqk_sb = big.tile([128, KT * CQK], BF16, tag="wqk")    # 8KB/p
        wv_sb = big.tile([128, KT * CV], BF16, tag="wv")       # 4KB/p
        wp_sb = big.tile([128, 2 * C], BF16, tag="wp")         # 4KB/p
        # qk_t layout: co blocks 0,1 = q (head pairs 0/1, 2/3), 2,3 = k.
        # Within a co block: even head on partitions 0-63, odd on 64-127.
        qk_sb = big.tile([128, 4 * T], BF16, tag="qk")         # 16KB/p
        # v_ext layout per (t16, head): 128 cols = [ones | 63 pad | v 64ch],
        # so av gets denom at PSUM partition 0 and channels at partition 64.
        v_sb = big.tile([128, NT16 * (H_LOC * 128)], BF16, tag="v")  # 16KB/p
        attn_sb = big.tile([128, 2 * T], BF16, tag="attn")     # 8KB/p
        cn_sb = big.tile([128, NCONST], F32, tag="consts")

        bqk_sb = cn_sb[:, CO_BQK:CO_BQK + 4]
        bvbc_sb = cn_sb[:, CO_BV:CO_BV + CV]
        tri_sb = big.tile([128, 128], BF16, tag="trib")

        # ---- input DMAs over four parallel rings (SP + Activation HWDGE,
        # Vector + GpSimd SWDGE). consts/tri first (tiny, needed by the
        # first PSUM drains); the first qk_block consumes (wqk k-tile,
        # xt0 k-tile) pairs in ascending k order, so quarter-granularity
        # sub-DMAs let the PE start before the whole chunk lands.
        # xt chunk q covers columns [q*4096, (q+1)*4096) = tq block q
        # (chunk-major layout: col = tq*4096 + k*512 + t).
        CH = KT * 512  # 4096 cols per tq chunk

        # Rings measured at ~157GB/s (SP / Activation HWDGE) and ~90GB/s
        # (GpSimd SWDGE). Each ring issues in NEED order: xt0 (k-ordered
        # quarters) + wqk first, then xt1..xt3 split across both HW rings,
        # wv/wp on the slow ring (needed mid-qkv0 / at first proj).
        nc.scalar.dma_start(cn_sb[:], consts[:])
        nc.scalar.dma_start(tri_sb[:], trib[:])
        nc.sync.dma_start(xt_sb[:, 0:1024], xt[:, 0:1024])
        nc.scalar.dma_start(wqk_sb[:, 0:2048], wqk[:, 0:2048])
        nc.sync.dma_start(xt_sb[:, 1024:2048], xt[:, 1024:2048])
        nc.gpsimd.dma_start(wv_sb[:], wv[:])
        nc.sync.dma_start(xt_sb[:, 2048:3072], xt[:, 2048:3072])
        nc.scalar.dma_start(wqk_sb[:, 2048:4096], wqk[:, 2048:4096])
        nc.sync.dma_start(xt_sb[:, 3072:4096], xt[:, 3072:4096])
        # xt1 split three ways so it lands well before qkv_step(1)
        nc.sync.dma_start(xt_sb[:, CH:CH + 1536], xt[:, CH:CH + 1536])
        nc.scalar.dma_start(xt_sb[:, CH + 1536:CH + 3072], xt[:, CH + 1536:CH + 3072])
        nc.gpsimd.dma_start(xt_sb[:, CH + 3072:2 * CH], xt[:, CH + 3072:2 * CH])
        nc.sync.dma_start(xt_sb[:, 2 * CH:2 * CH + 2048], xt[:, 2 * CH:2 * CH + 2048])
        nc.scalar.dma_start(xt_sb[:, 2 * CH + 2048:3 * CH], xt[:, 2 * CH + 2048:3 * CH])
        nc.gpsimd.dma_start(wp_sb[:], wp[:])
        nc.sync.dma_start(xt_sb[:, 3 * CH:3 * CH + 2048], xt[:, 3 * CH:3 * CH + 2048])
        nc.scalar.dma_start(xt_sb[:, 3 * CH + 2048:4 * CH], xt[:, 3 * CH + 2048:4 * CH])

        # ---- PE warmup on a DVE-initialized tile (no DMA dependency):
        # garbage matmuls into the recycled "mm" psum bufs keep the PE busy
        # from t~0 while inputs stream in, so the HAM activity window trips
        # and real matmuls start at 2.4GHz instead of 1.2GHz.
        wu_sb = big.tile([128, 512], BF16, tag="wu")
        nc.vector.memset(wu_sb[:], 0.5)

        def filler(n):
            for i in range(n):
                pw = psum.tile([128, 512], F32, tag="mm", bufs=2)
                nc.tensor.matmul(pw[:], wu_sb[:, 0:128], wu_sb[:],
                                 start=True, stop=True)

        filler(14)

        # ---- QKV projection ----
        def qk_block(co, tq):
            # qk_t[co*128:(co+1)*128, tq*512:(tq+1)*512]
            p = psum.tile([128, 512], F32, tag="mm", bufs=2)
            for k in range(KT):
                nc.tensor.matmul(p[:],
                                 wqk_sb[:, k * CQK + co * 128: k * CQK + (co + 1) * 128],
                                 xt_sb[:, tq * 4096 + k * 512: tq * 4096 + (k + 1) * 512],
                                 start=(k == 0), stop=(k == KT - 1))
            nc.vector.tensor_scalar_add(qk_sb[:, co * T + tq * 512: co * T + (tq + 1) * 512],
                                        p[:], bqk_sb[:, co:co + 1])

        def v_block_pair(t16a, t16b):
            tiles = [t for t in (t16a, t16b) if t is not None]
            ps = []
            for t16 in tiles:
                p = psum.tile([128, CV], F32, tag="mm", bufs=2, name=f"vp{t16}")
                ps.append(p)
            for k in range(KT):
                for p, t16 in zip(ps, tiles):
                    nc.tensor.matmul(p[:],
                                     xt_sb[:, (t16 // 4) * 4096 + k * 512 + (t16 % 4) * 128:
                                            (t16 // 4) * 4096 + k * 512 + (t16 % 4 + 1) * 128],
                                     wv_sb[:, k * CV:(k + 1) * CV],
                                     start=(k == 0), stop=(k == KT - 1))
            for p, t16 in zip(ps, tiles):
                out3 = v_sb[:, t16 * 512:(t16 + 1) * 512].rearrange("p (h d) -> p h d", d=128)[:, :, 64:128]
                in3 = p[:].rearrange("p (h d) -> p h d", d=64)
                b3 = bvbc_sb.rearrange("p (h d) -> p h d", d=64)
                nc.vector.tensor_add(out3, in3, b3)

        # ones + pad columns of v_ext via strided memsets on DVE (no DMA
        # dependency; emitted before any v write so the v tiles' other
        # columns never wait on them).
        nc.vector.memset(v_sb[:].rearrange("p (n d) -> p n d", d=128)[:, :, 0:1], 1.0)
        nc.vector.memset(v_sb[:].rearrange("p (n d) -> p n d", d=128)[:, :, 1:64], 0.0)

        def qkv_step(tq):
            qk_block(0, tq)
            qk_block(2, tq)
            v_block_pair(4 * tq, 4 * tq + 1)
            v_block_pair(4 * tq + 2, 4 * tq + 3)
            qk_block(1, tq)
            qk_block(3, tq)

        # ---- attention ----
        # One head PAIR per call: h0 = 2*hp (partitions 0-63), h1 = 2*hp+1
        # (64-127). Per key-tile kt the two S^T matmuls are K=64 row-tiles
        # at tile_position (0,0)/(64,0) (auto-derived) and run concurrently.
        # Diagonal blocks: m = kt - 4*tqb >= 0 means keys overlap queries;
        # live column range is [128*m, 512).
        def attn_pair(hp, tqb):
            nkt = 4 * (tqb + 1)
            av0 = psum.tile([128, 512], F32, tag="av0", bufs=1)
            av1 = psum.tile([128, 512], F32, tag="av1", bufs=1)
            for kt in range(nkt):
                m = kt - 4 * tqb
                c0 = max(m, 0) * 128
                s = psum.tile([128, 1024], F32, tag="s", bufs=2)
                e = work.tile([128, 1024], BF16, tag="e", bufs=2)
                for j in range(2):
                    h = 2 * hp + j
                    lo, hi = 64 * j, 64 * j + 64
                    nc.tensor.matmul(
                        s[:, j * 512 + c0: (j + 1) * 512],
                        qk_sb[lo:hi, (2 + hp) * T + kt * 128: (2 + hp) * T + (kt + 1) * 128],
                        qk_sb[lo:hi, hp * T + tqb * 512 + c0: hp * T + (tqb + 1) * 512],
                        start=True, stop=True)
                # exp via ScalarE (no max subtraction: |scores| <= ~8 here,
                # exp is safe in fp32). Restricted to live columns.
                if m < 0:
                    nc.scalar.activation(e[:], s[:], AF.Exp, scale=SCALE)
                else:  # diagonal: exp only each head's live columns
                    nc.scalar.activation(e[:, c0:512], s[:, c0:512],
                                         AF.Exp, scale=SCALE)
                    nc.scalar.activation(e[:, 512 + c0:1024], s[:, 512 + c0:1024],
                                         AF.Exp, scale=SCALE)
                if m >= 0:  # diagonal block: triangular mask, both heads
                    mc = 128 * m
                    nc.vector.tensor_mul(e[:, mc: mc + 128],
                                         e[:, mc: mc + 128], tri_sb[:])
                    nc.vector.tensor_mul(e[:, 512 + mc: 512 + mc + 128],
                                         e[:, 512 + mc: 512 + mc + 128], tri_sb[:])
                for j, av in ((0, av0), (1, av1)):
                    h = 2 * hp + j
                    nc.tensor.matmul(
                        av[:, c0:512],
                        v_sb[:, kt * 512 + h * 128: kt * 512 + (h + 1) * 128],
                        e[:, j * 512 + c0: (j + 1) * 512],
                        start=(kt == 0), stop=(kt == nkt - 1))
            # normalize: attn[:, cols] = av[64:128] * (1/av[0]).
            # av[0] is the denominator row at PSUM partition 0 (the custom
            # DVE reciprocal misreads nonzero partition offsets).
            for j, av in ((0, av0), (1, av1)):
                recipf = work.tile([1, 512], F32, tag=f"recipf{j}", bufs=2)
                nc.vector.reciprocal_approx_fast(recipf[:], av[0:1, :])
                bcs = work.tile([64, 512], F32, tag=f"bcs{j}")
                nc.gpsimd.partition_broadcast(bcs[:], recipf[:])
                nc.vector.tensor_mul(
                    attn_sb[64 * j:64 * j + 64, hp * T + tqb * 512: hp * T + (tqb + 1) * 512],
                    av[64:128, :], bcs[:])

        def proj_block(t16):
            ysb = work.tile([128, C], BF16, tag="y")
            for n in range(2):
                p = psum.tile([128, 512], F32, tag="mm", bufs=2)
                for kc in range(2):
                    nc.tensor.matmul(p[:],
                                     attn_sb[:, kc * T + t16 * 128: kc * T + (t16 + 1) * 128],
                                     wp_sb[:, kc * C + n * 512: kc * C + (n + 1) * 512],
                                     start=(kc == 0), stop=(kc == 1))
                # PSUM->SBUF drain split between ScalarE and DVE (GpSimd
                # cannot access PSUM); b_proj is folded in on the host.
                if n == 0:
                    nc.scalar.copy(ysb[:, n * 512:(n + 1) * 512], p[:])
                else:
                    nc.vector.tensor_copy(ysb[:, n * 512:(n + 1) * 512], p[:])
            # y DMAs alternate between the two HWDGE rings
            eng = nc.sync if t16 % 2 == 0 else nc.scalar
            eng.dma_start(y[t16 * 128:(t16 + 1) * 128, :], ysb[:])

        # Software pipeline: QKV for tq+1 is spliced between attention pairs
        # of tq so the PE has dense independent work while ScalarE runs exp.
        # tqb=0 is special: xt chunk 1 is still in flight, so both attention
        # pairs run first (their data is ready) and fillers bridge the
        # remaining DMA wait so the HAM clock gate never re-throttles.
        qkv_step(0)
        attn_pair(0, 0)
        attn_pair(1, 0)
        filler(6)
        qkv_step(1)
        for tqb in range(1, NTQ):
            nxt = tqb + 1
            prv = tqb - 1
            attn_pair(0, tqb)
            if nxt < NTQ:
                qk_block(0, nxt)
                qk_block(2, nxt)
            proj_block(4 * prv + 0)
            proj_block(4 * prv + 1)
            if nxt < NTQ:
                v_block_pair(4 * nxt, 4 * nxt + 1)
                v_block_pair(4 * nxt + 2, 4 * nxt + 3)
            attn_pair(1, tqb)
            if nxt < NTQ:
                qk_block(1, nxt)
                qk_block(3, nxt)
            proj_block(4 * prv + 2)
            proj_block(4 * prv + 3)
        for t16 in range(4 * 3, 4 * 4):
            proj_block(t16)

    nc.compile()
    return nc


def _get_nc():
    global _cached_nc
    if _cached_nc is None:
        _cached_nc = _build()
    return _cached_nc


def make_in_maps(x, w_attn, b_attn, w_proj, b_proj):
    x = np.asarray(x, np.float32)
    w_attn = np.asarray(w_attn, np.float32)
    b_attn = np.asarray(b_attn, np.float32)
    w_proj = np.asarray(w_proj, np.float32)
    bf = ml_dtypes.bfloat16
    tri = np.triu(np.ones((128, 128), np.float32))
    in_maps = []
    for core in range(N_CORES):
        b, hg = core // 4, core % 4
        cs = slice(hg * 256, (hg + 1) * 256)
        wqk_c = np.ascontiguousarray(
            np.concatenate([w_attn[:, cs], w_attn[:, 1024 + hg * 256:1024 + (hg + 1) * 256]],
                           axis=1)).astype(bf)
        bqk_vec = np.concatenate([b_attn[cs], b_attn[1024 + hg * 256:1024 + (hg + 1) * 256]])
        consts = np.zeros((128, NCONST), np.float32)
        consts[:, CO_BQK:CO_BQK + 4] = bqk_vec.reshape(4, 128).T
        consts[:, CO_BV:CO_BV + CV] = np.broadcast_to(
            b_attn[2048 + hg * 256:2048 + (hg + 1) * 256], (128, 256))
        # chunk-major xt: xtc[p, q*4096 + k*512 + t] = x[b, q*512 + t, k*128 + p]
        xtc = np.ascontiguousarray(
            x[b].reshape(4, 512, 8, 128).transpose(3, 0, 2, 1).reshape(128, KT * T))
        wv_c = w_attn[:, 2048 + hg * 256:2048 + (hg + 1) * 256]
        in_maps.append({
            "xt": xtc.astype(bf),
            "wqk": np.ascontiguousarray(
                wqk_c.reshape(8, 128, CQK).transpose(1, 0, 2).reshape(128, KT * CQK)),
            "wv": np.ascontiguousarray(
                wv_c.reshape(8, 128, CV).transpose(1, 0, 2).reshape(128, KT * CV)).astype(bf),
            "wp": np.ascontiguousarray(
                w_proj[cs, :].reshape(2, 128, C).transpose(1, 0, 2).reshape(128, 2 * C)).astype(bf),
            "consts": consts,
            "trib": tri.astype(bf),
        })
    return in_maps


def kernel(x, w_attn, b_attn, w_proj, b_proj):
    in_maps = make_in_maps(x, w_attn, b_attn, w_proj, b_proj)
    nc = _get_nc()
    res = run_bass_kernel_spmd(nc, in_maps, core_ids=list(range(N_CORES)))
    b_proj = np.asarray(b_proj, np.float32)
    y = np.zeros((B, T, C), np.float32)
    for core in range(N_CORES):
        y[core // 4] += np.asarray(res.results[core]["y"], dtype=np.float32)
    y += b_proj
    return y
